# revision 32
# baseline (speedup 1.0000x reference)
"""Trainium2 Bass kernel for nn_DensityVQC (batched 2-qubit VQC Z-expectation).

Algebra
-------
The reference builds rho_b = conj(psi_b) psi_b^T (note: transpose of the
standard density matrix), evolves rho' = U rho U^dag and returns
tr(rho' Z0) with Z0 = diag(1,1,-1,-1).  This collapses to a per-row
quadratic form: with V = conj(U) (the transposed-rho convention flips the
conjugation) and phi = V psi,

    out_b = |phi_0|^2 + |phi_1|^2 - |phi_2|^2 - |phi_3|^2
          = 2 * || C psi_b ||^2 - ||psi_b||^2        (C = V[0:2, :], U unitary)
          = || A r_b + B m_b ||^2 - 1                (inputs are unit-norm)

with real 4x4 matrices A = sqrt(2)*[Re C; Im C], B = sqrt(2)*[-Im C; Re C].
So the device kernel is: per batch row (r, m in R^4), compute w = A r + B m,
then out = sum(w^2) - 1.

Device mapping (per core, pure data parallel over 8 cores)
----------------------------------------------------------
Host-side marshalling interleaves r and m into ONE component-major tensor:
partition p = 8*g + c holds component c (r0..r3,m0..m3) of state-group g;
column n carries 16 states.  One [128,64] block-diagonal stationary W
(W[8g+c, 4g+j] = P[j,c], P = [A|B]) computes all four phi components of 16
groups in a single full-rate float32r matmul per 512-column supertile --
no PSUM accumulation pair and only one weight set for the whole projection.

PE array column-tiling (tile_position) packs results without padded weight
variants: the even supertile of a pair lands on PSUM rows 0:64, the odd on
rows 64:128 of the same bank, so one Square per PAIR ([128,512]) instead of
two.  The [128,32] group-sum stationary reduces a squared pair into rows
32q:32q+32 of a shared output bank (tile_position=(0,32q)), so one
PSUM->SBUF copy (with the -1 fold) covers four pairs.  Squares alternate
ACT/DVE; output stores ride the GpSimd SWDGE queue so the mid-stream store
never queues behind input chunks on the two HWDGE input rings.
"""

import sys
import numpy as np

if "/opt/trn_rl_repo" not in sys.path:
    sys.path.insert(0, "/opt/trn_rl_repo")

import concourse.bass as bass
import concourse.tile as tile
from concourse import bacc, mybir
from concourse import bass_utils
from concourse.tile_rust import add_dep_helper

N_CORES = 8
BSZ = 1_048_576
BC = BSZ // N_CORES            # 131072 rows per core
NCOL = BC // 16                # 8192 component-major free columns
CCOLS = 96                     # DMA'd const cols: Wproj [128,64] + zred [128,32]
CPAD = 416                     # on-SBUF zero-padded const block width
N_PAIRS = 8                    # pairs of 512-col supertiles
# PE clock management: ~3us of continuous PE activity ramps the clock from
# 1.2 GHz to 2.4 GHz; a power budget then allows roughly 7-12us of full
# speed before HAM clamps to half duty.  A front warm-up burst ramps the
# clock before the first data chunk lands so the pair pipeline (3 matmuls
# per 1.74us chunk cadence) runs at full speed through the stream and tail.
# Warm-up burst: 416-col f32r matmuls on the const block, gated on the
# const DMA.  Empirically this exact shape ramps the clock earliest (full
# speed ~2.5us after the burst ends); earlier/smaller bursts on memset
# zeros paradoxically delay the full-speed grant.
N_WARM = 10
F32 = mybir.dt.float32
F32R = mybir.dt.float32r
N_LAYERS = 6


def _circuit_unitary(ry, rz):
    """4x4 circuit unitary, float64 mirror of reference._circuit_unitary."""
    ry = np.asarray(ry, dtype=np.float64)
    rz = np.asarray(rz, dtype=np.float64)
    cnot = np.array(
        [[1, 0, 0, 0], [0, 1, 0, 0], [0, 0, 0, 1], [0, 0, 1, 0]],
        dtype=np.complex128,
    )

    def _ry(th):
        c, s = np.cos(th / 2), np.sin(th / 2)
        return np.array([[c, -s], [s, c]], dtype=np.complex128)

    def _rz(th):
        return np.diag([np.exp(-0.5j * th), np.exp(0.5j * th)])

    u = np.eye(4, dtype=np.complex128)
    for l in range(ry.shape[0]):
        ry_full = np.kron(_ry(ry[l, 0]), _ry(ry[l, 1]))
        rz_full = np.kron(_rz(rz[l, 0]), _rz(rz[l, 1]))
        u = cnot @ (rz_full @ (ry_full @ u))
    return u


def _host_consts(ry_params, rz_params):
    u = _circuit_unitary(ry_params, rz_params)
    c = np.conj(u)[0:2, :]
    a = np.sqrt(2.0) * np.vstack([c.real, c.imag])     # 4x4, w = A r + B m
    b = np.sqrt(2.0) * np.vstack([-c.imag, c.real])
    p = np.concatenate([a, b], axis=1).astype(np.float32)   # [4, 8]
    # Wproj[8g+c, 4g+j] = P[j, c]: the one projection stationary.  Matmul
    # PSUM writes cannot target a partition offset (walrus rejects nonzero
    # tile positions), so the odd/even supertile placement uses overlapping
    # 128-col slices of one zero-padded block: [Z64 | Wproj | Z64] gives
    # Wlo = cst[:, 64:192] = [Wproj | 0] (rows 0:64) and
    # Whi = cst[:, 0:128]  = [0 | Wproj] (rows 64:128), accumulated in PSUM.
    wproj = np.zeros((128, 64), dtype=np.float32)
    for g in range(16):
        wproj[8 * g : 8 * g + 8, 4 * g : 4 * g + 4] = p.T
    # zred[64h+4g+j, 16h+g] = 1.0: per-state sum of the 4 squared components
    # of a squared pair (even sup on rows 0:64, odd on 64:128).  Padded the
    # same way: [Z96 | zred | Z96]; zq[q] = slice [288-32q : 416-32q] puts
    # the group-sums on output rows 32q:32q+32.
    zred = np.zeros((128, 32), dtype=np.float32)
    for h in range(2):
        for g in range(16):
            zred[64 * h + 4 * g : 64 * h + 4 * g + 4, 16 * h + g] = 1.0
    return np.concatenate([wproj, zred], axis=1)       # [128, 96]


def _to_component_major(u8):
    """u8 [BC,8] f32 -> [128, NCOL]: tile[8g+c, n] = u8[16n+g, c]."""
    return np.ascontiguousarray(
        u8.reshape(NCOL, 16, 8).transpose(1, 2, 0).reshape(128, NCOL)
    )


def _from_out(y):
    """y [128, 1024] -> [BC].  Row 32q+16h+g, col 512B+n holds the value for
    state b = 16*(512*st + n) + g with st = 2*(4B+q)+h."""
    return np.ascontiguousarray(
        y.reshape(4, 2, 16, 2, 512).transpose(3, 0, 1, 4, 2)
    ).reshape(BC)


def _build_program():
    nc = bacc.Bacc("TRN2", target_bir_lowering=False, debug=False)
    # Data laid out chunk-contiguous in DRAM: chunk k = [128, 1024] with 4 KiB
    # per-partition lines (the fastest measured descriptor shape).
    cst_d = nc.dram_tensor("cst", [128, CCOLS], F32R, kind="ExternalInput")
    ut_d = nc.dram_tensor("ut", [N_PAIRS, 128, 1024], F32R,
                          kind="ExternalInput")
    out_d = nc.dram_tensor("out", [128, 1024], F32, kind="ExternalOutput")
    # Everything (loads AND stores) runs on the single SP HWDGE ring; the
    # Activation HWDGE and Pool SWDGE queues are never used.  Dropping their
    # declarations shrinks the runtime's per-ring teardown drain (32 fewer
    # rings to poll at NEFF exit, ~100 ns each).
    nc.m.queues = [
        q for q in nc.m.queues
        if q.name not in ("qActDynamicHW", "qPoolDynamic")
    ]
    nc.hwdge_engines = type(nc.hwdge_engines)([mybir.EngineType.SP])

    with tile.TileContext(nc) as tc:
        with (
            tc.tile_pool(name="io", bufs=1) as iopool,
            tc.tile_pool(name="work", bufs=4) as wpool,
            tc.tile_pool(name="psum", bufs=1, space=bass.MemorySpace.PSUM) as ppool,
        ):
            # SBUF layout: [0:CPAD) zero-padded const block, [CPAD:) data.
            ut_t = iopool.tile([128, CPAD + NCOL], F32R, name="ut_t")
            out_sb = iopool.tile([128, 1024], F32, name="out_sb")
            wlo = ut_t[:, 64:192]     # [Wproj | 0] -> phi rows 0:64
            whi = ut_t[:, 0:128]      # [0 | Wproj] -> phi rows 64:128
            zq = [ut_t[:, 288 - 32 * q : 416 - 32 * q] for q in range(4)]

            # Zero padding built on device (3 memsets on the idle Pool
            # engine) so the const DMA carries only the 96 real columns.
            # Bitcast to uint32: the Memset ISA op rejects float32r APs.
            nc.gpsimd.memset(ut_t[:, 0:64].bitcast(mybir.dt.uint32), 0)
            nc.gpsimd.memset(ut_t[:, 128:288].bitcast(mybir.dt.uint32), 0)
            nc.gpsimd.memset(ut_t[:, 320:416].bitcast(mybir.dt.uint32), 0)

            # --- all DMA on the single SP HWDGE ring (one InstDMACopy
            # already spreads across all 16 SDMA engines; one ring keeps
            # chunk arrival strictly in-order).
            dma_prev = [None]

            def dma(dst_ap, src_ap):
                d = nc.sync.dma_start(dst_ap, src_ap)
                if dma_prev[0] is not None:
                    add_dep_helper(d.ins, dma_prev[0].ins, sync=False,
                                   reason="q")
                dma_prev[0] = d

            dma(ut_t[:, 64:128], cst_d.ap()[:, 0:64])    # Wproj
            dma(ut_t[:, 288:320], cst_d.ap()[:, 64:96])  # zred
            for t in range(N_PAIRS):
                dst = CPAD + 1024 * t
                if t < N_PAIRS - 1:
                    dma(ut_t[:, dst:dst + 1024], ut_d.ap()[t])
                else:
                    # Last chunk in halves: the final supertile's compute
                    # chain starts half a chunk earlier.
                    dma(ut_t[:, dst:dst + 512], ut_d.ap()[t][:, 0:512])
                    dma(ut_t[:, dst + 512:dst + 1024],
                        ut_d.ap()[t][:, 512:1024])

            if N_WARM:
                warm = ppool.tile([128, 512], F32, name="warm", bufs=1)
                for w in range(N_WARM):
                    nc.tensor.matmul(warm[:, 0:CPAD], wlo, ut_t[:, 0:CPAD],
                                     start=True, stop=True)

            phis = [None] * N_PAIRS
            sqs = [None] * N_PAIRS
            obank = [None, None]

            def proj(t):
                phi = ppool.tile([128, 512], F32, name="phi", bufs=4)
                phis[t] = phi
                ce = CPAD + 1024 * t
                nc.tensor.matmul(phi[:], wlo, ut_t[:, ce:ce + 512],
                                 start=True, stop=False)
                nc.tensor.matmul(phi[:], whi, ut_t[:, ce + 512:ce + 1024],
                                 start=False, stop=True)

            def square(t):
                # ACT only: DVE TensorTensor cannot read two PSUM operands.
                s_t = wpool.tile([128, 512], F32R, name="s")
                sqs[t] = s_t
                nc.scalar.activation(
                    s_t[:], phis[t][:], mybir.ActivationFunctionType.Square
                )

            def reduce(t):
                q, b = t % 4, t // 4
                if q == 0:
                    obank[b] = ppool.tile([128, 512], F32, name="ob", bufs=2)
                nc.tensor.matmul(obank[b][:], zq[q], sqs[t][:],
                                 start=(q == 0), stop=(q == 3))

            def drain(b):
                # PSUM -> SBUF with the -1 fold (DVE, keeping ACT free for
                # squares), then store (rides the tail of the SP ring).
                cs = bass.ds(512 * b, 512)
                nc.vector.tensor_scalar_add(out_sb[:, cs], obank[b][:], -1.0)
                dma(out_d.ap()[:, cs], out_sb[:, cs])

            # Two-deep stagger: reduce(t-2) is emitted after proj(t), so by
            # the time the in-order PE reaches it, square(t-2) has long
            # finished and the PE never stalls on the ACT engine.
            proj(0)
            proj(1)
            square(0)
            square(1)
            for t in range(2, N_PAIRS - 1):
                proj(t)
                reduce(t - 2)
                square(t)
                if t - 2 == 3:
                    drain(0)

            # --- pair 7, split into 256-col half-pipelines so the final
            # square/reduce/drain/store chain after the last chunk lands is
            # half as deep, and the very last store is only 128 KiB (its
            # completion receipt gates the teardown sweeps).
            t7 = N_PAIRS - 1
            phi7 = ppool.tile([128, 512], F32, name="phi", bufs=4)
            phis[t7] = phi7
            s7 = wpool.tile([128, 512], F32R, name="s")
            ce = CPAD + 1024 * t7
            for h0 in (0, 256):
                nc.tensor.matmul(phi7[:, h0:h0 + 256], wlo,
                                 ut_t[:, ce + h0:ce + h0 + 256],
                                 start=True, stop=False)
                nc.tensor.matmul(phi7[:, h0:h0 + 256], whi,
                                 ut_t[:, ce + 512 + h0:ce + 512 + h0 + 256],
                                 start=False, stop=True)
            nc.scalar.activation(s7[:, 0:256], phi7[:, 0:256],
                                 mybir.ActivationFunctionType.Square)
            reduce(t7 - 2)
            reduce(t7 - 1)
            nc.scalar.activation(s7[:, 256:512], phi7[:, 256:512],
                                 mybir.ActivationFunctionType.Square)
            ob1 = obank[1]
            nc.tensor.matmul(ob1[:, 0:256], zq[3], s7[:, 0:256],
                             start=False, stop=False, skip_group_check=True)
            nc.tensor.matmul(ob1[:, 256:512], zq[3], s7[:, 256:512],
                             start=False, stop=True, skip_group_check=True)
            # Half-drains on separate engines, half-stores back to back.
            nc.vector.tensor_scalar_add(out_sb[:, 512:768], ob1[:, 0:256],
                                        -1.0)
            dma(out_d.ap()[:, 512:768], out_sb[:, 512:768])
            nc.scalar.activation(out_sb[:, 768:1024], ob1[:, 256:512],
                                 mybir.ActivationFunctionType.Copy, bias=-1.0)
            dma(out_d.ap()[:, 768:1024], out_sb[:, 768:1024])
    nc.compile()
    return nc


_PROG_CACHE = None


def _get_program():
    global _PROG_CACHE
    if _PROG_CACHE is None:
        _PROG_CACHE = _build_program()
    return _PROG_CACHE


def _run(ry_params, rz_params, states_real, states_imag, **hw_kwargs):
    consts = _host_consts(ry_params, rz_params)
    states_real = np.ascontiguousarray(states_real, dtype=np.float32)
    states_imag = np.ascontiguousarray(states_imag, dtype=np.float32)
    in_maps = []
    for k in range(N_CORES):
        sl = slice(k * BC, (k + 1) * BC)
        u8 = np.concatenate([states_real[sl], states_imag[sl]], axis=1)
        tile_cm = _to_component_major(u8)              # [128, 8192]
        chunks = np.ascontiguousarray(
            tile_cm.reshape(128, N_PAIRS, 1024).transpose(1, 0, 2)
        )                                              # [8, 128, 1024]
        in_maps.append({"cst": consts, "ut": chunks})
    nc = _get_program()
    res = bass_utils.run_bass_kernel_spmd(
        nc, in_maps, core_ids=list(range(N_CORES)), **hw_kwargs
    )
    out = np.concatenate(
        [_from_out(res.results[k]["out"]) for k in range(N_CORES)]
    ).astype(np.float32)
    return out, res


def kernel(ry_params, rz_params, states_real, states_imag):
    out, _ = _run(ry_params, rz_params, states_real, states_imag)
    return out


# revision 34
# speedup vs baseline: 1.0674x; 1.0674x over previous
"""Trainium2 Bass kernel for nn_DensityVQC (batched 2-qubit VQC Z-expectation).

Algebra
-------
The reference builds rho_b = conj(psi_b) psi_b^T (note: transpose of the
standard density matrix), evolves rho' = U rho U^dag and returns
tr(rho' Z0) with Z0 = diag(1,1,-1,-1).  This collapses to a per-row
quadratic form: with V = conj(U) (the transposed-rho convention flips the
conjugation) and phi = V psi,

    out_b = |phi_0|^2 + |phi_1|^2 - |phi_2|^2 - |phi_3|^2
          = 2 * || C psi_b ||^2 - ||psi_b||^2        (C = V[0:2, :], U unitary)
          = || A r_b + B m_b ||^2 - 1                (inputs are unit-norm)

with real 4x4 matrices A = sqrt(2)*[Re C; Im C], B = sqrt(2)*[-Im C; Re C].
So the device kernel is: per batch row (r, m in R^4), compute w = A r + B m,
then out = sum(w^2) - 1.

Device mapping (per core, pure data parallel over 8 cores)
----------------------------------------------------------
Host-side marshalling interleaves r and m into ONE component-major tensor:
partition p = 8*g + c holds component c (r0..r3,m0..m3) of state-group g;
column n carries 16 states.  One [128,64] block-diagonal stationary W
(W[8g+c, 4g+j] = P[j,c], P = [A|B]) computes all four phi components of 16
groups in a single full-rate float32r matmul per 512-column supertile --
no PSUM accumulation pair and only one weight set for the whole projection.

PE array column-tiling (tile_position) packs results without padded weight
variants: the even supertile of a pair lands on PSUM rows 0:64, the odd on
rows 64:128 of the same bank, so one Square per PAIR ([128,512]) instead of
two.  The [128,32] group-sum stationary reduces a squared pair into rows
32q:32q+32 of a shared output bank (tile_position=(0,32q)), so one
PSUM->SBUF copy (with the -1 fold) covers four pairs.  Squares alternate
ACT/DVE; output stores ride the GpSimd SWDGE queue so the mid-stream store
never queues behind input chunks on the two HWDGE input rings.
"""

import sys
import numpy as np

if "/opt/trn_rl_repo" not in sys.path:
    sys.path.insert(0, "/opt/trn_rl_repo")

import concourse.bass as bass
import concourse.tile as tile
from concourse import bacc, mybir
from concourse import bass_utils
from concourse.tile_rust import add_dep_helper

N_CORES = 8
BSZ = 1_048_576
BC = BSZ // N_CORES            # 131072 rows per core
NCOL = BC // 16                # 8192 component-major free columns
CCOLS = 96                     # DMA'd const cols: Wproj [128,64] + zred [128,32]
CPAD = 416                     # on-SBUF zero-padded const block width
N_PAIRS = 8                    # pairs of 512-col supertiles
# PE clock management: ~3us of continuous PE activity ramps the clock from
# 1.2 GHz to 2.4 GHz; a power budget then allows roughly 7-12us of full
# speed before HAM clamps to half duty.  A front warm-up burst ramps the
# clock before the first data chunk lands so the pair pipeline (3 matmuls
# per 1.74us chunk cadence) runs at full speed through the stream and tail.
# Warms multiply memset-zero SBUF columns so they depend only on the (fast,
# engine-signalled) memsets -- not on the const DMA and its ~1.5us HBM
# completion receipt.  Sized so the burst stays busy until the first pair's
# chunk semaphore (~12.5us): a gap between warm-up and the pair pipeline
# resets the clock ramp.
N_WARM = 22
F32 = mybir.dt.float32
F32R = mybir.dt.float32r
N_LAYERS = 6


def _circuit_unitary(ry, rz):
    """4x4 circuit unitary, float64 mirror of reference._circuit_unitary."""
    ry = np.asarray(ry, dtype=np.float64)
    rz = np.asarray(rz, dtype=np.float64)
    cnot = np.array(
        [[1, 0, 0, 0], [0, 1, 0, 0], [0, 0, 0, 1], [0, 0, 1, 0]],
        dtype=np.complex128,
    )

    def _ry(th):
        c, s = np.cos(th / 2), np.sin(th / 2)
        return np.array([[c, -s], [s, c]], dtype=np.complex128)

    def _rz(th):
        return np.diag([np.exp(-0.5j * th), np.exp(0.5j * th)])

    u = np.eye(4, dtype=np.complex128)
    for l in range(ry.shape[0]):
        ry_full = np.kron(_ry(ry[l, 0]), _ry(ry[l, 1]))
        rz_full = np.kron(_rz(rz[l, 0]), _rz(rz[l, 1]))
        u = cnot @ (rz_full @ (ry_full @ u))
    return u


def _host_consts(ry_params, rz_params):
    u = _circuit_unitary(ry_params, rz_params)
    c = np.conj(u)[0:2, :]
    a = np.sqrt(2.0) * np.vstack([c.real, c.imag])     # 4x4, w = A r + B m
    b = np.sqrt(2.0) * np.vstack([-c.imag, c.real])
    p = np.concatenate([a, b], axis=1).astype(np.float32)   # [4, 8]
    # Wproj[8g+c, 4g+j] = P[j, c]: the one projection stationary.  Matmul
    # PSUM writes cannot target a partition offset (walrus rejects nonzero
    # tile positions), so the odd/even supertile placement uses overlapping
    # 128-col slices of one zero-padded block: [Z64 | Wproj | Z64] gives
    # Wlo = cst[:, 64:192] = [Wproj | 0] (rows 0:64) and
    # Whi = cst[:, 0:128]  = [0 | Wproj] (rows 64:128), accumulated in PSUM.
    wproj = np.zeros((128, 64), dtype=np.float32)
    for g in range(16):
        wproj[8 * g : 8 * g + 8, 4 * g : 4 * g + 4] = p.T
    # zred[64h+4g+j, 16h+g] = 1.0: per-state sum of the 4 squared components
    # of a squared pair (even sup on rows 0:64, odd on 64:128).  Padded the
    # same way: [Z96 | zred | Z96]; zq[q] = slice [288-32q : 416-32q] puts
    # the group-sums on output rows 32q:32q+32.
    zred = np.zeros((128, 32), dtype=np.float32)
    for h in range(2):
        for g in range(16):
            zred[64 * h + 4 * g : 64 * h + 4 * g + 4, 16 * h + g] = 1.0
    return np.concatenate([wproj, zred], axis=1)       # [128, 96]


def _to_component_major(u8):
    """u8 [BC,8] f32 -> [128, NCOL]: tile[8g+c, n] = u8[16n+g, c]."""
    return np.ascontiguousarray(
        u8.reshape(NCOL, 16, 8).transpose(1, 2, 0).reshape(128, NCOL)
    )


def _from_out(y):
    """y [128, 1024] -> [BC].  Row 32q+16h+g, col 512B+n holds the value for
    state b = 16*(512*st + n) + g with st = 2*(4B+q)+h."""
    return np.ascontiguousarray(
        y.reshape(4, 2, 16, 2, 512).transpose(3, 0, 1, 4, 2)
    ).reshape(BC)


def _build_program():
    nc = bacc.Bacc("TRN2", target_bir_lowering=False, debug=False)
    # Data laid out chunk-contiguous in DRAM: chunk k = [128, 1024] with 4 KiB
    # per-partition lines (the fastest measured descriptor shape).
    cst_d = nc.dram_tensor("cst", [128, CCOLS], F32R, kind="ExternalInput")
    ut_d = nc.dram_tensor("ut", [N_PAIRS, 128, 1024], F32R,
                          kind="ExternalInput")
    out_d = nc.dram_tensor("out", [128, 1024], F32, kind="ExternalOutput")
    # Everything (loads AND stores) runs on the single SP HWDGE ring; the
    # Activation HWDGE and Pool SWDGE queues are never used.  Dropping their
    # declarations shrinks the runtime's per-ring teardown drain (32 fewer
    # rings to poll at NEFF exit, ~100 ns each).
    nc.m.queues = [
        q for q in nc.m.queues
        if q.name not in ("qActDynamicHW", "qPoolDynamic")
    ]
    nc.hwdge_engines = type(nc.hwdge_engines)([mybir.EngineType.SP])

    with tile.TileContext(nc) as tc:
        with (
            tc.tile_pool(name="io", bufs=1) as iopool,
            tc.tile_pool(name="work", bufs=4) as wpool,
            tc.tile_pool(name="psum", bufs=1, space=bass.MemorySpace.PSUM) as ppool,
        ):
            # SBUF layout: [0:CPAD) zero-padded const block, [CPAD:) data.
            ut_t = iopool.tile([128, CPAD + NCOL], F32R, name="ut_t")
            out_sb = iopool.tile([128, 1024], F32, name="out_sb")
            wlo = ut_t[:, 64:192]     # [Wproj | 0] -> phi rows 0:64
            whi = ut_t[:, 0:128]      # [0 | Wproj] -> phi rows 64:128
            zq = [ut_t[:, 288 - 32 * q : 416 - 32 * q] for q in range(4)]

            # Zero padding built on device (3 memsets on the idle Pool
            # engine) so the const DMA carries only the 96 real columns.
            # Bitcast to uint32: the Memset ISA op rejects float32r APs.
            nc.gpsimd.memset(ut_t[:, 0:64].bitcast(mybir.dt.uint32), 0)
            nc.gpsimd.memset(ut_t[:, 128:288].bitcast(mybir.dt.uint32), 0)
            nc.gpsimd.memset(ut_t[:, 320:416].bitcast(mybir.dt.uint32), 0)

            # --- all DMA on the single SP HWDGE ring (one InstDMACopy
            # already spreads across all 16 SDMA engines; one ring keeps
            # chunk arrival strictly in-order).
            dma_prev = [None]

            def dma(dst_ap, src_ap):
                d = nc.sync.dma_start(dst_ap, src_ap)
                if dma_prev[0] is not None:
                    add_dep_helper(d.ins, dma_prev[0].ins, sync=False,
                                   reason="q")
                dma_prev[0] = d

            dma(ut_t[:, 64:128], cst_d.ap()[:, 0:64])    # Wproj
            dma(ut_t[:, 288:320], cst_d.ap()[:, 64:96])  # zred
            for t in range(N_PAIRS):
                dst = CPAD + 1024 * t
                if t < N_PAIRS - 1:
                    dma(ut_t[:, dst:dst + 1024], ut_d.ap()[t])
                else:
                    # Last chunk in halves: the final supertile's compute
                    # chain starts half a chunk earlier.
                    dma(ut_t[:, dst:dst + 512], ut_d.ap()[t][:, 0:512])
                    dma(ut_t[:, dst + 512:dst + 1024],
                        ut_d.ap()[t][:, 512:1024])

            if N_WARM:
                warm = ppool.tile([128, 512], F32, name="warm", bufs=1)
                for w in range(N_WARM):
                    nc.tensor.matmul(warm[:, 0:160], ut_t[:, 128:256],
                                     ut_t[:, 128:288], start=True, stop=True)

            phis = [None] * N_PAIRS
            sqs = [None] * N_PAIRS
            obank = [None, None]

            def proj(t):
                phi = ppool.tile([128, 512], F32, name="phi", bufs=4)
                phis[t] = phi
                ce = CPAD + 1024 * t
                nc.tensor.matmul(phi[:], wlo, ut_t[:, ce:ce + 512],
                                 start=True, stop=False)
                nc.tensor.matmul(phi[:], whi, ut_t[:, ce + 512:ce + 1024],
                                 start=False, stop=True)

            def square(t):
                # ACT only: DVE TensorTensor cannot read two PSUM operands.
                s_t = wpool.tile([128, 512], F32R, name="s")
                sqs[t] = s_t
                nc.scalar.activation(
                    s_t[:], phis[t][:], mybir.ActivationFunctionType.Square
                )

            def reduce(t):
                q, b = t % 4, t // 4
                if q == 0:
                    obank[b] = ppool.tile([128, 512], F32, name="ob", bufs=2)
                nc.tensor.matmul(obank[b][:], zq[q], sqs[t][:],
                                 start=(q == 0), stop=(q == 3))

            def drain(b):
                # PSUM -> SBUF with the -1 fold (DVE, keeping ACT free for
                # squares), then store (rides the tail of the SP ring).
                cs = bass.ds(512 * b, 512)
                nc.vector.tensor_scalar_add(out_sb[:, cs], obank[b][:], -1.0)
                dma(out_d.ap()[:, cs], out_sb[:, cs])

            # Two-deep stagger: reduce(t-2) is emitted after proj(t), so by
            # the time the in-order PE reaches it, square(t-2) has long
            # finished and the PE never stalls on the ACT engine.
            proj(0)
            proj(1)
            square(0)
            square(1)
            for t in range(2, N_PAIRS - 1):
                proj(t)
                reduce(t - 2)
                square(t)
                if t - 2 == 3:
                    drain(0)

            # --- pair 7, split into 256-col half-pipelines so the final
            # square/reduce/drain/store chain after the last chunk lands is
            # half as deep, and the very last store is only 128 KiB (its
            # completion receipt gates the teardown sweeps).
            t7 = N_PAIRS - 1
            phi7 = ppool.tile([128, 512], F32, name="phi", bufs=4)
            phis[t7] = phi7
            s7 = wpool.tile([128, 512], F32R, name="s")
            ce = CPAD + 1024 * t7
            for h0 in (0, 256):
                nc.tensor.matmul(phi7[:, h0:h0 + 256], wlo,
                                 ut_t[:, ce + h0:ce + h0 + 256],
                                 start=True, stop=False)
                nc.tensor.matmul(phi7[:, h0:h0 + 256], whi,
                                 ut_t[:, ce + 512 + h0:ce + 512 + h0 + 256],
                                 start=False, stop=True)
            nc.scalar.activation(s7[:, 0:256], phi7[:, 0:256],
                                 mybir.ActivationFunctionType.Square)
            reduce(t7 - 2)
            reduce(t7 - 1)
            nc.scalar.activation(s7[:, 256:512], phi7[:, 256:512],
                                 mybir.ActivationFunctionType.Square)
            ob1 = obank[1]
            nc.tensor.matmul(ob1[:, 0:256], zq[3], s7[:, 0:256],
                             start=False, stop=False, skip_group_check=True)
            nc.tensor.matmul(ob1[:, 256:512], zq[3], s7[:, 256:512],
                             start=False, stop=True, skip_group_check=True)
            # Half-drains on separate engines, half-stores back to back.
            nc.vector.tensor_scalar_add(out_sb[:, 512:768], ob1[:, 0:256],
                                        -1.0)
            dma(out_d.ap()[:, 512:768], out_sb[:, 512:768])
            nc.scalar.activation(out_sb[:, 768:1024], ob1[:, 256:512],
                                 mybir.ActivationFunctionType.Copy, bias=-1.0)
            dma(out_d.ap()[:, 768:1024], out_sb[:, 768:1024])
    nc.compile()
    return nc


_PROG_CACHE = None


def _get_program():
    global _PROG_CACHE
    if _PROG_CACHE is None:
        _PROG_CACHE = _build_program()
    return _PROG_CACHE


def _run(ry_params, rz_params, states_real, states_imag, **hw_kwargs):
    consts = _host_consts(ry_params, rz_params)
    states_real = np.ascontiguousarray(states_real, dtype=np.float32)
    states_imag = np.ascontiguousarray(states_imag, dtype=np.float32)
    in_maps = []
    for k in range(N_CORES):
        sl = slice(k * BC, (k + 1) * BC)
        u8 = np.concatenate([states_real[sl], states_imag[sl]], axis=1)
        tile_cm = _to_component_major(u8)              # [128, 8192]
        chunks = np.ascontiguousarray(
            tile_cm.reshape(128, N_PAIRS, 1024).transpose(1, 0, 2)
        )                                              # [8, 128, 1024]
        in_maps.append({"cst": consts, "ut": chunks})
    nc = _get_program()
    res = bass_utils.run_bass_kernel_spmd(
        nc, in_maps, core_ids=list(range(N_CORES)), **hw_kwargs
    )
    out = np.concatenate(
        [_from_out(res.results[k]["out"]) for k in range(N_CORES)]
    ).astype(np.float32)
    return out, res


def kernel(ry_params, rz_params, states_real, states_imag):
    out, _ = _run(ry_params, rz_params, states_real, states_imag)
    return out
